# revision 7
# baseline (speedup 1.0000x reference)
"""Trainium2 Bass kernel for nn_AttentionModel (B=4, C=128, H=W=64).

Self-attention over spatial positions with 1x1-conv QKV projections and a
gamma-scaled residual:
    out = gamma * softmax(Q K / sqrt(C)) V + x

Sharding: data-parallel over batch (4 samples) x sequence-parallel over
query rows (2 halves of N=4096) = 8 NeuronCores. Each core holds the full
[C,C] weights, computes K/V for its whole sample, and the attention output
for its 2048 query rows.

Per-core algorithm (all matmuls bf16 with fp32 PSUM accumulate):
  QT[c,n] = WqT.T @ xf (+bq)   (pre-scaled by 1/sqrt(C) on host)
  K [c,m] = WkT.T @ xf (+bk)
  V [m,c] = xf_chunk.T @ WvT    (32 chunks of 128 rows; bv folded at end)
  per 512-wide group of query rows n, per 128-chunk of key index m:
    S^T[m,n] = K_chunk.T @ QT_group        (PE)
    P^T      = exp(S^T)                    (ACT, bf16 out)
    rowsum  += ones.T @ P^T                (PE, PSUM accumulate, bcast to 128p)
    pvacc   += V_chunk.T @ P^T             (PE, PSUM accumulate)
  recip = exp(-ln(rowsum))                 (ACT; Ln+Exp share a table set)
  out = (pvacc * recip + bv) * gamma + x   (DVE)

The softmax skips max-subtraction: energies are ~N(0,1) here, exp is safe.
"""

import numpy as np
import ml_dtypes

import concourse.bass as bass
import concourse.mybir as mybir
import concourse.tile as tile
from concourse import bacc
from concourse.bass_utils import run_bass_kernel_spmd

B, C, H, W = 4, 128, 64, 64
N = H * W            # 4096 spatial positions
NCORES = 8
RQ = N * B // NCORES  # 2048 query rows per core
NG = 512             # query-row group width (PSUM bank)
MC = 128             # key-chunk width (PE contraction)
F32 = mybir.dt.float32
BF16 = mybir.dt.bfloat16
AF = mybir.ActivationFunctionType


def build_bass():
    nc = bacc.Bacc("TRN2", target_bir_lowering=False, debug=False,
                   num_devices=NCORES)

    xf = nc.dram_tensor("xf", [C, N], BF16, kind="ExternalInput")
    xq = nc.dram_tensor("xq", [C, RQ], BF16, kind="ExternalInput")
    xr = nc.dram_tensor("xr", [C, RQ], F32, kind="ExternalInput")
    wqt = nc.dram_tensor("wqt", [C, C], BF16, kind="ExternalInput")
    wkt = nc.dram_tensor("wkt", [C, C], BF16, kind="ExternalInput")
    wvt = nc.dram_tensor("wvt", [C, C], BF16, kind="ExternalInput")
    bq = nc.dram_tensor("bq", [C, 1], F32, kind="ExternalInput")
    bk = nc.dram_tensor("bk", [C, 1], F32, kind="ExternalInput")
    bv = nc.dram_tensor("bv", [C, 1], F32, kind="ExternalInput")
    gm = nc.dram_tensor("gm", [C, 1], F32, kind="ExternalInput")
    out = nc.dram_tensor("out", [C, RQ], F32, kind="ExternalOutput")

    n_mc = N // MC      # 32 key chunks
    n_g = RQ // NG      # 4 query groups

    with tile.TileContext(nc) as tc:
        with tc.tile_pool(name="const", bufs=1) as cp:
            xf_t = cp.tile([C, N], BF16, tag="xf")
            xq_t = cp.tile([C, RQ], BF16, tag="xq")
            xr_t = cp.tile([C, RQ], F32, tag="xr")
            wq_t = cp.tile([C, C], BF16, tag="wq")
            wk_t = cp.tile([C, C], BF16, tag="wk")
            wv_t = cp.tile([C, C], BF16, tag="wv")
            bq_t = cp.tile([C, 1], F32, tag="bq")
            bk_t = cp.tile([C, 1], F32, tag="bk")
            bv_t = cp.tile([C, 1], F32, tag="bv")
            gm_t = cp.tile([C, 1], F32, tag="gm")
            ones_t = cp.tile([C, C], BF16, tag="ones")
            kk_t = cp.tile([C, N], BF16, tag="kk")
            qt_t = cp.tile([C, RQ], BF16, tag="qt")
            vv_t = cp.tile([C, n_mc, MC], BF16, tag="vv")

            nc.sync.dma_start(xf_t[:], xf[:])
            nc.sync.dma_start(xq_t[:], xq[:])
            nc.sync.dma_start(xr_t[:], xr[:])
            nc.sync.dma_start(wq_t[:], wqt[:])
            nc.sync.dma_start(wk_t[:], wkt[:])
            nc.sync.dma_start(wv_t[:], wvt[:])
            nc.sync.dma_start(bq_t[:], bq[:])
            nc.sync.dma_start(bk_t[:], bk[:])
            nc.sync.dma_start(bv_t[:], bv[:])
            nc.sync.dma_start(gm_t[:], gm[:])
            nc.vector.memset(ones_t[:], 1.0)

            # ---- Phase 1: projections ----
            with tc.tile_pool(name="ph1psum", bufs=4,
                              space=bass.MemorySpace.PSUM) as pp1:
                for j in range(N // NG):  # K over the full sample
                    ps = pp1.tile([C, NG], F32, tag="p1")
                    nc.tensor.matmul(ps[:], wk_t[:], xf_t[:, bass.ts(j, NG)],
                                     start=True, stop=True)
                    nc.vector.tensor_scalar_add(
                        out=kk_t[:, bass.ts(j, NG)], in0=ps[:], scalar1=bk_t[:])
                for j in range(RQ // NG):  # Q over my query rows only
                    ps = pp1.tile([C, NG], F32, tag="p1")
                    nc.tensor.matmul(ps[:], wq_t[:], xq_t[:, bass.ts(j, NG)],
                                     start=True, stop=True)
                    nc.vector.tensor_scalar_add(
                        out=qt_t[:, bass.ts(j, NG)], in0=ps[:], scalar1=bq_t[:])
                for mc in range(n_mc):  # V in [m, c] layout
                    ps = pp1.tile([C, MC], F32, tag="p1v")
                    nc.tensor.matmul(ps[:], xf_t[:, bass.ts(mc, MC)], wv_t[:],
                                     start=True, stop=True)
                    nc.scalar.activation(vv_t[:, mc, :], ps[:], AF.Copy)

            # ---- Phase 2: attention ----
            with (
                tc.tile_pool(name="stp", bufs=3,
                             space=bass.MemorySpace.PSUM) as stp,
                tc.tile_pool(name="rsp", bufs=2,
                             space=bass.MemorySpace.PSUM) as rsp,
                tc.tile_pool(name="pvp", bufs=2,
                             space=bass.MemorySpace.PSUM) as pvp,
                tc.tile_pool(name="ptp", bufs=6) as ptp,
                tc.tile_pool(name="fin", bufs=3) as fin,
            ):
                for g in range(n_g):
                    rs_ps = rsp.tile([C, NG], F32, tag="rs")
                    pv_ps = pvp.tile([C, NG], F32, tag="pv")
                    for mc in range(n_mc):
                        st_ps = stp.tile([C, NG], F32, tag="st")
                        nc.tensor.matmul(st_ps[:], kk_t[:, bass.ts(mc, MC)],
                                         qt_t[:, bass.ts(g, NG)],
                                         start=True, stop=True)
                        pt = ptp.tile([C, NG], BF16, tag="pt")
                        nc.scalar.activation(pt[:], st_ps[:], AF.Exp)
                        nc.tensor.matmul(rs_ps[:], ones_t[:], pt[:],
                                         start=(mc == 0), stop=(mc == n_mc - 1))
                        nc.tensor.matmul(pv_ps[:], vv_t[:, mc, :], pt[:],
                                         start=(mc == 0), stop=(mc == n_mc - 1))
                    lg = fin.tile([C, NG], F32, tag="lg")
                    nc.scalar.activation(lg[:], rs_ps[:], AF.Ln)
                    rb = fin.tile([C, NG], F32, tag="rb")
                    nc.scalar.activation(rb[:], lg[:], AF.Exp, scale=-1.0)
                    o1 = fin.tile([C, NG], F32, tag="o1")
                    nc.vector.tensor_mul(o1[:], pv_ps[:], rb[:])
                    o2 = fin.tile([C, NG], F32, tag="o2")
                    nc.vector.tensor_scalar(
                        out=o2[:], in0=o1[:], scalar1=bv_t[:], scalar2=gm_t[:],
                        op0=mybir.AluOpType.add, op1=mybir.AluOpType.mult)
                    o3 = fin.tile([C, NG], F32, tag="o3")
                    nc.vector.tensor_add(o3[:], o2[:], xr_t[:, bass.ts(g, NG)])
                    nc.sync.dma_start(out[:, bass.ts(g, NG)], o3[:])

    nc.compile()
    return nc


_NC_CACHE = None


def _get_nc():
    global _NC_CACHE
    if _NC_CACHE is None:
        _NC_CACHE = build_bass()
    return _NC_CACHE


def make_in_maps(x, Wq, bq, Wk, bk, Wv, bv, gamma):
    x = np.asarray(x, dtype=np.float32)
    Wq = np.asarray(Wq, dtype=np.float32)
    Wk = np.asarray(Wk, dtype=np.float32)
    Wv = np.asarray(Wv, dtype=np.float32)
    bq = np.asarray(bq, dtype=np.float32)
    bk = np.asarray(bk, dtype=np.float32)
    bv = np.asarray(bv, dtype=np.float32)
    gamma = np.asarray(gamma, dtype=np.float32)

    scale = np.float32(1.0 / np.sqrt(C))
    xf = x.reshape(B, C, N)
    wqt = np.ascontiguousarray((Wq * scale).T).astype(ml_dtypes.bfloat16)
    wkt = np.ascontiguousarray(Wk.T).astype(ml_dtypes.bfloat16)
    wvt = np.ascontiguousarray(Wv.T).astype(ml_dtypes.bfloat16)
    bq_s = (bq * scale).reshape(C, 1)
    bk_s = bk.reshape(C, 1).copy()
    bv_s = bv.reshape(C, 1).copy()
    gm_s = np.full((C, 1), gamma.reshape(-1)[0], dtype=np.float32)

    in_maps = []
    for core in range(NCORES):
        b, h = core // 2, core % 2
        xslice = np.ascontiguousarray(xf[b][:, h * RQ:(h + 1) * RQ])
        in_maps.append({
            "xf": xf[b].astype(ml_dtypes.bfloat16),
            "xq": xslice.astype(ml_dtypes.bfloat16),
            "xr": xslice,
            "wqt": wqt, "wkt": wkt, "wvt": wvt,
            "bq": bq_s, "bk": bk_s, "bv": bv_s, "gm": gm_s,
        })
    return in_maps


def assemble(results):
    out = np.empty((B, C, N), dtype=np.float32)
    for core in range(NCORES):
        b, h = core // 2, core % 2
        out[b][:, h * RQ:(h + 1) * RQ] = results[core]["out"]
    return out.reshape(B, C, H, W)


def run(inputs: dict, trace: bool = False, tmpdir: str | None = None):
    nc = _get_nc()
    in_maps = make_in_maps(**inputs)
    res = run_bass_kernel_spmd(nc, in_maps, core_ids=list(range(NCORES)),
                               trace=trace, tmpdir=tmpdir)
    return assemble(res.results), res


def kernel(**inputs) -> np.ndarray:
    out, _ = run(inputs, trace=False)
    return out


# revision 8
# speedup vs baseline: 1.0677x; 1.0677x over previous
"""Trainium2 Bass kernel for nn_AttentionModel (B=4, C=128, H=W=64).

Self-attention over spatial positions with 1x1-conv QKV projections and a
gamma-scaled residual:
    out = gamma * softmax(Q K / sqrt(C)) V + x

Sharding: data-parallel over batch (4 samples) x sequence-parallel over
query rows (2 halves of N=4096) = 8 NeuronCores. Each core holds the full
[C,C] weights, computes K/V for its whole sample, and the attention output
for its 2048 query rows.

Per-core algorithm (all matmuls bf16 with fp32 PSUM accumulate):
  QT[c,n] = WqT.T @ xf (+bq)   (pre-scaled by 1/sqrt(C) on host)
  K [c,m] = WkT.T @ xf (+bk)
  V [m,c] = xf_chunk.T @ WvT    (32 chunks of 128 rows; bv folded at end)
  per 512-wide group of query rows n, per 128-chunk of key index m:
    S^T[m,n] = K_chunk.T @ QT_group        (PE)
    P^T      = exp(S^T)                    (ACT, bf16 out)
    rowsum  += ones.T @ P^T                (PE, PSUM accumulate, bcast to 128p)
    pvacc   += V_chunk.T @ P^T             (PE, PSUM accumulate)
  recip = exp(-ln(rowsum))                 (ACT; Ln+Exp share a table set)
  out = (pvacc * recip + bv) * gamma + x   (DVE)

The softmax skips max-subtraction: energies are ~N(0,1) here, exp is safe.
"""

import numpy as np
import ml_dtypes

import concourse.bass as bass
import concourse.mybir as mybir
import concourse.tile as tile
from concourse import bacc
from concourse.bass_utils import run_bass_kernel_spmd

B, C, H, W = 4, 128, 64, 64
N = H * W            # 4096 spatial positions
NCORES = 8
RQ = N * B // NCORES  # 2048 query rows per core
NG = 512             # query-row group width (PSUM bank)
MC = 128             # key-chunk width (PE contraction)
F32 = mybir.dt.float32
BF16 = mybir.dt.bfloat16
AF = mybir.ActivationFunctionType


def build_bass():
    nc = bacc.Bacc("TRN2", target_bir_lowering=False, debug=False,
                   num_devices=NCORES)

    xf = nc.dram_tensor("xf", [C, N], BF16, kind="ExternalInput")
    xq = nc.dram_tensor("xq", [C, RQ], BF16, kind="ExternalInput")
    xr = nc.dram_tensor("xr", [C, RQ], F32, kind="ExternalInput")
    wqt = nc.dram_tensor("wqt", [C, C], BF16, kind="ExternalInput")
    wkt = nc.dram_tensor("wkt", [C, C], BF16, kind="ExternalInput")
    wvt = nc.dram_tensor("wvt", [C, C], BF16, kind="ExternalInput")
    bq = nc.dram_tensor("bq", [C, 1], F32, kind="ExternalInput")
    bk = nc.dram_tensor("bk", [C, 1], F32, kind="ExternalInput")
    bv = nc.dram_tensor("bv", [C, 1], F32, kind="ExternalInput")
    gm = nc.dram_tensor("gm", [C, 1], F32, kind="ExternalInput")
    out = nc.dram_tensor("out", [C, RQ], F32, kind="ExternalOutput")

    n_mc = N // MC      # 32 key chunks
    n_g = RQ // NG      # 4 query groups

    with tile.TileContext(nc) as tc:
        with tc.tile_pool(name="const", bufs=1) as cp:
            xf_t = cp.tile([C, N], BF16, tag="xf")
            xq_t = cp.tile([C, RQ], BF16, tag="xq")
            xr_t = cp.tile([C, RQ], F32, tag="xr")
            wq_t = cp.tile([C, C], BF16, tag="wq")
            wk_t = cp.tile([C, C], BF16, tag="wk")
            wv_t = cp.tile([C, C], BF16, tag="wv")
            bq_t = cp.tile([C, 1], F32, tag="bq")
            bk_t = cp.tile([C, 1], F32, tag="bk")
            bv_t = cp.tile([C, 1], F32, tag="bv")
            gm_t = cp.tile([C, 1], F32, tag="gm")
            ones_t = cp.tile([C, C], BF16, tag="ones")
            kk_t = cp.tile([C, N], BF16, tag="kk")
            qt_t = cp.tile([C, RQ], BF16, tag="qt")
            vv_t = cp.tile([C, n_mc, MC], BF16, tag="vv")

            nc.sync.dma_start(xf_t[:], xf[:])
            nc.sync.dma_start(xq_t[:], xq[:])
            nc.sync.dma_start(xr_t[:], xr[:])
            nc.sync.dma_start(wq_t[:], wqt[:])
            nc.sync.dma_start(wk_t[:], wkt[:])
            nc.sync.dma_start(wv_t[:], wvt[:])
            nc.sync.dma_start(bq_t[:], bq[:])
            nc.sync.dma_start(bk_t[:], bk[:])
            nc.sync.dma_start(bv_t[:], bv[:])
            nc.sync.dma_start(gm_t[:], gm[:])
            nc.vector.memset(ones_t[:], 1.0)

            # ---- Phase 1: projections ----
            with tc.tile_pool(name="ph1psum", bufs=4,
                              space=bass.MemorySpace.PSUM) as pp1:
                for j in range(N // NG):  # K over the full sample
                    ps = pp1.tile([C, NG], F32, tag="p1")
                    nc.tensor.matmul(ps[:], wk_t[:], xf_t[:, bass.ts(j, NG)],
                                     start=True, stop=True)
                    nc.vector.tensor_scalar_add(
                        out=kk_t[:, bass.ts(j, NG)], in0=ps[:], scalar1=bk_t[:])
                for j in range(RQ // NG):  # Q over my query rows only
                    ps = pp1.tile([C, NG], F32, tag="p1")
                    nc.tensor.matmul(ps[:], wq_t[:], xq_t[:, bass.ts(j, NG)],
                                     start=True, stop=True)
                    nc.vector.tensor_scalar_add(
                        out=qt_t[:, bass.ts(j, NG)], in0=ps[:], scalar1=bq_t[:])
                for mc in range(n_mc):  # V in [m, c] layout
                    ps = pp1.tile([C, MC], F32, tag="p1v")
                    nc.tensor.matmul(ps[:], xf_t[:, bass.ts(mc, MC)], wv_t[:],
                                     start=True, stop=True)
                    nc.scalar.activation(vv_t[:, mc, :], ps[:], AF.Copy)

            # ---- Phase 2: attention ----
            # One 2048-wide query group. Per key-chunk mc: 4 S^T matmuls
            # fill two [C,1024] PSUM tiles, one exp per tile (ACT), 4 PV
            # matmuls accumulate into a 4-bank [C,2048] PSUM tile, and the
            # P^T chunk is accumulated (bf16) for the softmax row-sums —
            # even chunks on DVE, odd on GPSIMD so neither stalls ACT.
            NH = 1024  # exp granularity: two PSUM banks
            with (
                tc.tile_pool(name="stp", bufs=2,
                             space=bass.MemorySpace.PSUM) as stp,
                tc.tile_pool(name="pvp", bufs=1,
                             space=bass.MemorySpace.PSUM) as pvp,
                tc.tile_pool(name="ptp", bufs=5) as ptp,
                tc.tile_pool(name="accp", bufs=1) as accp,
                tc.tile_pool(name="fin", bufs=1) as fin,
            ):
                pv_ps = pvp.tile([C, RQ], F32, tag="pv")
                acc_d = accp.tile([C, RQ], BF16, tag="acc_d")
                acc_g = accp.tile([C, RQ], BF16, tag="acc_g")
                for mc in range(n_mc):
                    pts = []
                    for h in range(RQ // NH):
                        st_ps = stp.tile([C, NH], F32, tag="st")
                        for q in range(NH // NG):
                            nn = h * NH + q * NG
                            nc.tensor.matmul(
                                st_ps[:, bass.ts(q, NG)],
                                kk_t[:, bass.ts(mc, MC)],
                                qt_t[:, bass.ds(nn, NG)],
                                start=True, stop=True)
                        pt = ptp.tile([C, NH], BF16, tag="pt")
                        nc.scalar.activation(pt[:], st_ps[:], AF.Exp)
                        pts.append(pt)
                        for q in range(NH // NG):
                            nn = h * NH + q * NG
                            nc.tensor.matmul(
                                pv_ps[:, bass.ds(nn, NG)],
                                vv_t[:, mc, :], pt[:, bass.ts(q, NG)],
                                start=(mc == 0), stop=(mc == n_mc - 1))
                    eng = nc.vector if mc % 2 == 0 else nc.gpsimd
                    acc = acc_d if mc % 2 == 0 else acc_g
                    for h, pt in enumerate(pts):
                        if mc < 2:
                            eng.tensor_copy(acc[:, bass.ts(h, NH)], pt[:])
                        else:
                            eng.tensor_add(acc[:, bass.ts(h, NH)],
                                           acc[:, bass.ts(h, NH)], pt[:])

                # softmax denominators: merge the two accumulators, then a
                # ones-matmul reduces the 128 partition lanes in fp32
                acc_t = accp.tile([C, RQ], BF16, tag="acc_t")
                nc.vector.tensor_add(acc_t[:], acc_d[:], acc_g[:])
                rb = fin.tile([C, RQ], F32, tag="rb")
                for h in range(RQ // NH):
                    rs_ps = stp.tile([C, NH], F32, tag="st")
                    for q in range(NH // NG):
                        nn = h * NH + q * NG
                        nc.tensor.matmul(rs_ps[:, bass.ts(q, NG)], ones_t[:],
                                         acc_t[:, bass.ds(nn, NG)],
                                         start=True, stop=True)
                    nc.vector.reciprocal_approx_fast(
                        out=rb[:, bass.ts(h, NH)], in_=rs_ps[:])

                o1 = fin.tile([C, RQ], F32, tag="o1")
                nc.vector.tensor_mul(o1[:], pv_ps[:], rb[:])
                o2 = fin.tile([C, RQ], F32, tag="o2")
                nc.vector.tensor_scalar(
                    out=o2[:], in0=o1[:], scalar1=bv_t[:], scalar2=gm_t[:],
                    op0=mybir.AluOpType.add, op1=mybir.AluOpType.mult)
                o3 = fin.tile([C, RQ], F32, tag="o3")
                nc.vector.tensor_add(o3[:], o2[:], xr_t[:])
                nc.sync.dma_start(out[:], o3[:])

    nc.compile()
    return nc


_NC_CACHE = None


def _get_nc():
    global _NC_CACHE
    if _NC_CACHE is None:
        _NC_CACHE = build_bass()
    return _NC_CACHE


def make_in_maps(x, Wq, bq, Wk, bk, Wv, bv, gamma):
    x = np.asarray(x, dtype=np.float32)
    Wq = np.asarray(Wq, dtype=np.float32)
    Wk = np.asarray(Wk, dtype=np.float32)
    Wv = np.asarray(Wv, dtype=np.float32)
    bq = np.asarray(bq, dtype=np.float32)
    bk = np.asarray(bk, dtype=np.float32)
    bv = np.asarray(bv, dtype=np.float32)
    gamma = np.asarray(gamma, dtype=np.float32)

    scale = np.float32(1.0 / np.sqrt(C))
    xf = x.reshape(B, C, N)
    wqt = np.ascontiguousarray((Wq * scale).T).astype(ml_dtypes.bfloat16)
    wkt = np.ascontiguousarray(Wk.T).astype(ml_dtypes.bfloat16)
    wvt = np.ascontiguousarray(Wv.T).astype(ml_dtypes.bfloat16)
    bq_s = (bq * scale).reshape(C, 1)
    bk_s = bk.reshape(C, 1).copy()
    bv_s = bv.reshape(C, 1).copy()
    gm_s = np.full((C, 1), gamma.reshape(-1)[0], dtype=np.float32)

    in_maps = []
    for core in range(NCORES):
        b, h = core // 2, core % 2
        xslice = np.ascontiguousarray(xf[b][:, h * RQ:(h + 1) * RQ])
        in_maps.append({
            "xf": xf[b].astype(ml_dtypes.bfloat16),
            "xq": xslice.astype(ml_dtypes.bfloat16),
            "xr": xslice,
            "wqt": wqt, "wkt": wkt, "wvt": wvt,
            "bq": bq_s, "bk": bk_s, "bv": bv_s, "gm": gm_s,
        })
    return in_maps


def assemble(results):
    out = np.empty((B, C, N), dtype=np.float32)
    for core in range(NCORES):
        b, h = core // 2, core % 2
        out[b][:, h * RQ:(h + 1) * RQ] = results[core]["out"]
    return out.reshape(B, C, H, W)


def run(inputs: dict, trace: bool = False, tmpdir: str | None = None):
    nc = _get_nc()
    in_maps = make_in_maps(**inputs)
    res = run_bass_kernel_spmd(nc, in_maps, core_ids=list(range(NCORES)),
                               trace=trace, tmpdir=tmpdir)
    return assemble(res.results), res


def kernel(**inputs) -> np.ndarray:
    out, _ = run(inputs, trace=False)
    return out


# revision 9
# speedup vs baseline: 1.1157x; 1.0449x over previous
"""Trainium2 Bass kernel for nn_AttentionModel (B=4, C=128, H=W=64).

Self-attention over spatial positions with 1x1-conv QKV projections and a
gamma-scaled residual:
    out = gamma * softmax(Q K / sqrt(C)) V + x

Sharding: data-parallel over batch (4 samples) x sequence-parallel over
query rows (2 halves of N=4096) = 8 NeuronCores. Each core holds the full
[C,C] weights, computes K/V for its whole sample, and the attention output
for its 2048 query rows.

Per-core algorithm (all matmuls bf16 with fp32 PSUM accumulate):
  QT[c,n] = WqT.T @ xf (+bq)   (pre-scaled by 1/sqrt(C) on host)
  K [c,m] = WkT.T @ xf (+bk)
  V [m,c] = xf_chunk.T @ WvT    (32 chunks of 128 rows; bv folded at end)
  per 512-wide group of query rows n, per 128-chunk of key index m:
    S^T[m,n] = K_chunk.T @ QT_group        (PE)
    P^T      = exp(S^T)                    (ACT, bf16 out)
    rowsum  += ones.T @ P^T                (PE, PSUM accumulate, bcast to 128p)
    pvacc   += V_chunk.T @ P^T             (PE, PSUM accumulate)
  recip = exp(-ln(rowsum))                 (ACT; Ln+Exp share a table set)
  out = (pvacc * recip + bv) * gamma + x   (DVE)

The softmax skips max-subtraction: energies are ~N(0,1) here, exp is safe.
"""

import numpy as np
import ml_dtypes

import concourse.bass as bass
import concourse.mybir as mybir
import concourse.tile as tile
from concourse import bacc
from concourse.bass_utils import run_bass_kernel_spmd

B, C, H, W = 4, 128, 64, 64
N = H * W            # 4096 spatial positions
NCORES = 8
RQ = N * B // NCORES  # 2048 query rows per core
NG = 512             # query-row group width (PSUM bank)
MC = 128             # key-chunk width (PE contraction)
F32 = mybir.dt.float32
BF16 = mybir.dt.bfloat16
AF = mybir.ActivationFunctionType


def build_bass():
    nc = bacc.Bacc("TRN2", target_bir_lowering=False, debug=False,
                   num_devices=NCORES)

    xf = nc.dram_tensor("xf", [C, N], BF16, kind="ExternalInput")
    xq = nc.dram_tensor("xq", [C, RQ], BF16, kind="ExternalInput")
    xr = nc.dram_tensor("xr", [C, RQ], F32, kind="ExternalInput")
    wqt = nc.dram_tensor("wqt", [C, C], BF16, kind="ExternalInput")
    wkt = nc.dram_tensor("wkt", [C, C], BF16, kind="ExternalInput")
    wvt = nc.dram_tensor("wvt", [C, C], BF16, kind="ExternalInput")
    bq = nc.dram_tensor("bq", [C, 1], F32, kind="ExternalInput")
    bk = nc.dram_tensor("bk", [C, 1], F32, kind="ExternalInput")
    bvg = nc.dram_tensor("bvg", [C, 1], F32, kind="ExternalInput")
    gm = nc.dram_tensor("gm", [C, 1], F32, kind="ExternalInput")
    out = nc.dram_tensor("out", [C, RQ], F32, kind="ExternalOutput")

    n_mc = N // MC       # 32 key chunks
    NSG = 1024           # query supergroup width
    n_sg = RQ // NSG     # 2 supergroups

    with tile.TileContext(nc) as tc:
        with tc.tile_pool(name="const", bufs=1) as cp:
            xf_t = cp.tile([C, N], BF16, tag="xf")
            xq_t = cp.tile([C, RQ], BF16, tag="xq")
            xr_t = cp.tile([C, RQ], F32, tag="xr")
            wq_t = cp.tile([C, C], BF16, tag="wq")
            wk_t = cp.tile([C, C], BF16, tag="wk")
            wv_t = cp.tile([C, C], BF16, tag="wv")
            bq_t = cp.tile([C, 1], F32, tag="bq")
            bk_t = cp.tile([C, 1], F32, tag="bk")
            bvg_t = cp.tile([C, 1], F32, tag="bvg")
            gm_t = cp.tile([C, 1], F32, tag="gm")
            ones_t = cp.tile([C, C], BF16, tag="ones")
            kk_t = cp.tile([C, N], BF16, tag="kk")
            qt_t = cp.tile([C, RQ], BF16, tag="qt")
            vv_t = cp.tile([C, n_mc, MC], BF16, tag="vv")

            # Loads, most-urgent first: weights/biases (small), then xq
            # (feeds the Q projection), then xf in chunks so the K matmuls
            # release early. xr is only needed by the epilogue.
            nc.sync.dma_start(wq_t[:], wqt[:])
            nc.sync.dma_start(wk_t[:], wkt[:])
            nc.sync.dma_start(wv_t[:], wvt[:])
            nc.sync.dma_start(bq_t[:], bq[:])
            nc.sync.dma_start(bk_t[:], bk[:])
            nc.sync.dma_start(bvg_t[:], bvg[:])
            nc.sync.dma_start(gm_t[:], gm[:])
            nc.vector.memset(ones_t[:], 1.0)
            nc.sync.dma_start(xq_t[:], xq[:])
            NXF = N // 4
            for j in range(4):
                nc.sync.dma_start(xf_t[:, bass.ts(j, NXF)],
                                  xf[:, bass.ts(j, NXF)])
            nc.sync.dma_start(xr_t[:], xr[:])

            with (
                tc.tile_pool(name="stp", bufs=2,
                             space=bass.MemorySpace.PSUM) as stp,
                tc.tile_pool(name="pvp", bufs=1,
                             space=bass.MemorySpace.PSUM) as pvp,
                tc.tile_pool(name="vpp", bufs=2,
                             space=bass.MemorySpace.PSUM) as vpp,
                tc.tile_pool(name="ptp", bufs=4) as ptp,
                tc.tile_pool(name="accp", bufs=2) as accp,
                tc.tile_pool(name="fin", bufs=2) as fin,
            ):
                # Q projection first (gates the first S^T matmul), then K.
                for j in range(RQ // NG):
                    ps = vpp.tile([C, NG], F32, tag="vp")
                    nc.tensor.matmul(ps[:], wq_t[:], xq_t[:, bass.ts(j, NG)],
                                     start=True, stop=True)
                    nc.vector.tensor_scalar_add(
                        out=qt_t[:, bass.ts(j, NG)], in0=ps[:], scalar1=bq_t[:])
                for j in range(N // NG):
                    ps = vpp.tile([C, NG], F32, tag="vp")
                    nc.tensor.matmul(ps[:], wk_t[:], xf_t[:, bass.ts(j, NG)],
                                     start=True, stop=True)
                    nc.vector.tensor_scalar_add(
                        out=kk_t[:, bass.ts(j, NG)], in0=ps[:], scalar1=bk_t[:])

                for sg in range(n_sg):
                    pv_ps = pvp.tile([C, NSG], F32, tag="pv")
                    acc_d = accp.tile([C, NSG], BF16, tag="acc_d")
                    acc_g = accp.tile([C, NSG], BF16, tag="acc_g")
                    for mc in range(n_mc):
                        st_ps = stp.tile([C, NSG], F32, tag="st")
                        for q in range(NSG // NG):
                            nn = sg * NSG + q * NG
                            nc.tensor.matmul(
                                st_ps[:, bass.ts(q, NG)],
                                kk_t[:, bass.ts(mc, MC)],
                                qt_t[:, bass.ds(nn, NG)],
                                start=True, stop=True)
                        pt = ptp.tile([C, NSG], BF16, tag="pt")
                        nc.scalar.activation(pt[:], st_ps[:], AF.Exp)
                        if sg == 0:
                            # just-in-time V projection for this key chunk
                            vp = vpp.tile([C, MC], F32, tag="vp")
                            nc.tensor.matmul(vp[:], xf_t[:, bass.ts(mc, MC)],
                                             wv_t[:], start=True, stop=True)
                            nc.vector.tensor_copy(vv_t[:, mc, :], vp[:])
                        for q in range(NSG // NG):
                            nc.tensor.matmul(
                                pv_ps[:, bass.ts(q, NG)],
                                vv_t[:, mc, :], pt[:, bass.ts(q, NG)],
                                start=(mc == 0), stop=(mc == n_mc - 1))
                        eng = nc.vector if mc % 2 == 0 else nc.gpsimd
                        acc = acc_d if mc % 2 == 0 else acc_g
                        if mc < 2:
                            eng.tensor_copy(acc[:], pt[:])
                        else:
                            eng.tensor_add(acc[:], acc[:], pt[:])

                    # softmax denominators + fused epilogue for this sg
                    acc_t = accp.tile([C, NSG], BF16, tag="acc_t")
                    nc.vector.tensor_add(acc_t[:], acc_d[:], acc_g[:])
                    rs_ps = stp.tile([C, NSG], F32, tag="st")
                    for q in range(NSG // NG):
                        nc.tensor.matmul(rs_ps[:, bass.ts(q, NG)], ones_t[:],
                                         acc_t[:, bass.ts(q, NG)],
                                         start=True, stop=True)
                    rb = fin.tile([C, NSG], F32, tag="rb")
                    nc.vector.reciprocal_approx_fast(out=rb[:], in_=rs_ps[:])
                    t1 = fin.tile([C, NSG], F32, tag="t1")
                    nc.vector.scalar_tensor_tensor(
                        out=t1[:], in0=pv_ps[:], scalar=gm_t[:], in1=rb[:],
                        op0=mybir.AluOpType.mult, op1=mybir.AluOpType.mult)
                    o3 = fin.tile([C, NSG], F32, tag="o3")
                    nc.vector.scalar_tensor_tensor(
                        out=o3[:], in0=t1[:], scalar=bvg_t[:],
                        in1=xr_t[:, bass.ts(sg, NSG)],
                        op0=mybir.AluOpType.add, op1=mybir.AluOpType.add)
                    nc.sync.dma_start(out[:, bass.ts(sg, NSG)], o3[:])

    nc.compile()
    return nc


_NC_CACHE = None


def _get_nc():
    global _NC_CACHE
    if _NC_CACHE is None:
        _NC_CACHE = build_bass()
    return _NC_CACHE


def make_in_maps(x, Wq, bq, Wk, bk, Wv, bv, gamma):
    x = np.asarray(x, dtype=np.float32)
    Wq = np.asarray(Wq, dtype=np.float32)
    Wk = np.asarray(Wk, dtype=np.float32)
    Wv = np.asarray(Wv, dtype=np.float32)
    bq = np.asarray(bq, dtype=np.float32)
    bk = np.asarray(bk, dtype=np.float32)
    bv = np.asarray(bv, dtype=np.float32)
    gamma = np.asarray(gamma, dtype=np.float32)

    scale = np.float32(1.0 / np.sqrt(C))
    xf = x.reshape(B, C, N)
    wqt = np.ascontiguousarray((Wq * scale).T).astype(ml_dtypes.bfloat16)
    wkt = np.ascontiguousarray(Wk.T).astype(ml_dtypes.bfloat16)
    wvt = np.ascontiguousarray(Wv.T).astype(ml_dtypes.bfloat16)
    bq_s = (bq * scale).reshape(C, 1)
    bk_s = bk.reshape(C, 1).copy()
    g0 = np.float32(gamma.reshape(-1)[0])
    bvg_s = (bv.reshape(C, 1) * g0).astype(np.float32)
    gm_s = np.full((C, 1), g0, dtype=np.float32)

    in_maps = []
    for core in range(NCORES):
        b, h = core // 2, core % 2
        xslice = np.ascontiguousarray(xf[b][:, h * RQ:(h + 1) * RQ])
        in_maps.append({
            "xf": xf[b].astype(ml_dtypes.bfloat16),
            "xq": xslice.astype(ml_dtypes.bfloat16),
            "xr": xslice,
            "wqt": wqt, "wkt": wkt, "wvt": wvt,
            "bq": bq_s, "bk": bk_s, "bvg": bvg_s, "gm": gm_s,
        })
    return in_maps


def assemble(results):
    out = np.empty((B, C, N), dtype=np.float32)
    for core in range(NCORES):
        b, h = core // 2, core % 2
        out[b][:, h * RQ:(h + 1) * RQ] = results[core]["out"]
    return out.reshape(B, C, H, W)


def run(inputs: dict, trace: bool = False, tmpdir: str | None = None):
    nc = _get_nc()
    in_maps = make_in_maps(**inputs)
    res = run_bass_kernel_spmd(nc, in_maps, core_ids=list(range(NCORES)),
                               trace=trace, tmpdir=tmpdir)
    return assemble(res.results), res


def kernel(**inputs) -> np.ndarray:
    out, _ = run(inputs, trace=False)
    return out


# revision 10
# speedup vs baseline: 1.2623x; 1.1314x over previous
"""Trainium2 Bass kernel for nn_AttentionModel (B=4, C=128, H=W=64).

Self-attention over spatial positions with 1x1-conv QKV projections and a
gamma-scaled residual:
    out = gamma * softmax(Q K / sqrt(C)) V + x

Sharding: data-parallel over batch (4 samples) x sequence-parallel over
query rows (2 halves of N=4096) = 8 NeuronCores. Each core holds the full
[C,C] weights, computes K/V for its whole sample, and the attention output
for its 2048 query rows.

Per-core algorithm (all matmuls bf16 with fp32 PSUM accumulate):
  QT[c,n] = WqT.T @ xf (+bq)   (pre-scaled by 1/sqrt(C) on host)
  K [c,m] = WkT.T @ xf (+bk)
  V [m,c] = xf_chunk.T @ WvT    (32 chunks of 128 rows; bv folded at end)
  per 512-wide group of query rows n, per 128-chunk of key index m:
    S^T[m,n] = K_chunk.T @ QT_group        (PE)
    P^T      = exp(S^T)                    (ACT, bf16 out)
    rowsum  += ones.T @ P^T                (PE, PSUM accumulate, bcast to 128p)
    pvacc   += V_chunk.T @ P^T             (PE, PSUM accumulate)
  recip = exp(-ln(rowsum))                 (ACT; Ln+Exp share a table set)
  out = (pvacc * recip + bv) * gamma + x   (DVE)

The softmax skips max-subtraction: energies are ~N(0,1) here, exp is safe.
"""

import numpy as np
import ml_dtypes

import concourse.bass as bass
import concourse.mybir as mybir
import concourse.tile as tile
from concourse import bacc
from concourse.bass_utils import run_bass_kernel_spmd

B, C, H, W = 4, 128, 64, 64
N = H * W            # 4096 spatial positions
NCORES = 8
RQ = N * B // NCORES  # 2048 query rows per core
NG = 512             # query-row group width (PSUM bank)
MC = 128             # key-chunk width (PE contraction)
F32 = mybir.dt.float32
BF16 = mybir.dt.bfloat16
AF = mybir.ActivationFunctionType


def build_bass():
    nc = bacc.Bacc("TRN2", target_bir_lowering=False, debug=False,
                   num_devices=NCORES)

    xf = nc.dram_tensor("xf", [C, N], BF16, kind="ExternalInput")
    xq = nc.dram_tensor("xq", [C, RQ], BF16, kind="ExternalInput")
    xr = nc.dram_tensor("xr", [C, RQ], F32, kind="ExternalInput")
    wqt = nc.dram_tensor("wqt", [C, C], BF16, kind="ExternalInput")
    wkt = nc.dram_tensor("wkt", [C, C], BF16, kind="ExternalInput")
    wvt = nc.dram_tensor("wvt", [C, C], BF16, kind="ExternalInput")
    bq = nc.dram_tensor("bq", [C, 1], F32, kind="ExternalInput")
    bk = nc.dram_tensor("bk", [C, 1], F32, kind="ExternalInput")
    bvg = nc.dram_tensor("bvg", [C, 1], F32, kind="ExternalInput")
    gm = nc.dram_tensor("gm", [C, 1], F32, kind="ExternalInput")
    out = nc.dram_tensor("out", [C, RQ], F32, kind="ExternalOutput")

    n_mc = N // MC       # 32 key chunks
    NSG = 1024           # query supergroup width
    n_sg = RQ // NSG     # 2 supergroups

    with tile.TileContext(nc) as tc:
        with tc.tile_pool(name="const", bufs=1) as cp:
            xf_t = cp.tile([C, N], BF16, tag="xf")
            xq_t = cp.tile([C, RQ], BF16, tag="xq")
            xr_t = cp.tile([C, RQ], F32, tag="xr")
            wq_t = cp.tile([C, C], BF16, tag="wq")
            wk_t = cp.tile([C, C], BF16, tag="wk")
            wv_t = cp.tile([C, C], BF16, tag="wv")
            bq_t = cp.tile([C, 1], F32, tag="bq")
            bk_t = cp.tile([C, 1], F32, tag="bk")
            bvg_t = cp.tile([C, 1], F32, tag="bvg")
            gm_t = cp.tile([C, 1], F32, tag="gm")
            ones_t = cp.tile([C, C], BF16, tag="ones")
            kk_t = cp.tile([C, N], BF16, tag="kk")
            qt_t = cp.tile([C, RQ], BF16, tag="qt")
            vv_t = cp.tile([C, n_mc, MC], BF16, tag="vv")

            # Loads, most-urgent first: weights/biases (small), then xq
            # (feeds the Q projection), then xf in chunks so the K matmuls
            # release early. xr is only needed by the epilogue.
            nc.sync.dma_start(wq_t[:], wqt[:])
            nc.sync.dma_start(wk_t[:], wkt[:])
            nc.sync.dma_start(wv_t[:], wvt[:])
            nc.sync.dma_start(bq_t[:], bq[:])
            nc.sync.dma_start(bk_t[:], bk[:])
            nc.sync.dma_start(bvg_t[:], bvg[:])
            nc.sync.dma_start(gm_t[:], gm[:])
            nc.vector.memset(ones_t[:], 1.0)
            nc.sync.dma_start(xq_t[:], xq[:])
            NXF = N // 4
            for j in range(4):
                nc.sync.dma_start(xf_t[:, bass.ts(j, NXF)],
                                  xf[:, bass.ts(j, NXF)])
            nc.sync.dma_start(xr_t[:], xr[:])

            with (
                tc.tile_pool(name="stp", bufs=2,
                             space=bass.MemorySpace.PSUM) as stp,
                tc.tile_pool(name="pvp", bufs=1,
                             space=bass.MemorySpace.PSUM) as pvp,
                tc.tile_pool(name="vpp", bufs=2,
                             space=bass.MemorySpace.PSUM) as vpp,
                tc.tile_pool(name="ptp", bufs=8) as ptp,
                tc.tile_pool(name="accp", bufs=2) as accp,
                tc.tile_pool(name="fin", bufs=2) as fin,
            ):
                # Q projection first (gates the first S^T matmul), then K.
                for j in range(RQ // NG):
                    ps = vpp.tile([C, NG], F32, tag="vp")
                    nc.tensor.matmul(ps[:], wq_t[:], xq_t[:, bass.ts(j, NG)],
                                     start=True, stop=True)
                    nc.vector.tensor_scalar_add(
                        out=qt_t[:, bass.ts(j, NG)], in0=ps[:], scalar1=bq_t[:])
                for j in range(N // NG):
                    ps = vpp.tile([C, NG], F32, tag="vp")
                    nc.tensor.matmul(ps[:], wk_t[:], xf_t[:, bass.ts(j, NG)],
                                     start=True, stop=True)
                    nc.vector.tensor_scalar_add(
                        out=kk_t[:, bass.ts(j, NG)], in0=ps[:], scalar1=bk_t[:])

                for sg in range(n_sg):
                    pv_ps = pvp.tile([C, NSG], F32, tag="pv")
                    acc_d = accp.tile([C, NSG], BF16, tag="acc_d")
                    acc_g = accp.tile([C, NSG], BF16, tag="acc_g")
                    for mc in range(n_mc):
                        st_ps = stp.tile([C, NSG], F32, tag="st")
                        for q in range(NSG // NG):
                            nn = sg * NSG + q * NG
                            nc.tensor.matmul(
                                st_ps[:, bass.ts(q, NG)],
                                kk_t[:, bass.ts(mc, MC)],
                                qt_t[:, bass.ds(nn, NG)],
                                start=True, stop=True)
                        pt = ptp.tile([C, NSG], BF16, tag="pt")
                        nc.scalar.activation(pt[:], st_ps[:], AF.Exp)
                        if sg == 0:
                            # just-in-time V projection for this key chunk
                            vp = vpp.tile([C, MC], F32, tag="vp")
                            nc.tensor.matmul(vp[:], xf_t[:, bass.ts(mc, MC)],
                                             wv_t[:], start=True, stop=True)
                            nc.vector.tensor_copy(vv_t[:, mc, :], vp[:])
                        for q in range(NSG // NG):
                            nc.tensor.matmul(
                                pv_ps[:, bass.ts(q, NG)],
                                vv_t[:, mc, :], pt[:, bass.ts(q, NG)],
                                start=(mc == 0), stop=(mc == n_mc - 1))
                        on_gp = (mc % 3 == 1)
                        acc = acc_g if on_gp else acc_d
                        if mc < 2:
                            nc.vector.tensor_copy(acc[:], pt[:])
                        elif on_gp:
                            nc.gpsimd.tensor_add(acc[:], acc[:], pt[:])
                        else:
                            nc.vector.tensor_add(acc[:], acc[:], pt[:])

                    # softmax denominators: rowsum = ones.T @ acc_d + ones.T @ acc_g
                    rs_ps = stp.tile([C, NSG], F32, tag="st")
                    for q in range(NSG // NG):
                        nc.tensor.matmul(rs_ps[:, bass.ts(q, NG)], ones_t[:],
                                         acc_d[:, bass.ts(q, NG)],
                                         start=True, stop=False)
                        nc.tensor.matmul(rs_ps[:, bass.ts(q, NG)], ones_t[:],
                                         acc_g[:, bass.ts(q, NG)],
                                         start=False, stop=True)
                    rb = fin.tile([C, NSG], F32, tag="rb")
                    nc.vector.reciprocal_approx_fast(out=rb[:], in_=rs_ps[:])
                    t1 = fin.tile([C, NSG], F32, tag="t1")
                    nc.vector.scalar_tensor_tensor(
                        out=t1[:], in0=pv_ps[:], scalar=gm_t[:], in1=rb[:],
                        op0=mybir.AluOpType.mult, op1=mybir.AluOpType.mult)
                    o3 = fin.tile([C, NSG], F32, tag="o3")
                    nc.vector.scalar_tensor_tensor(
                        out=o3[:], in0=t1[:], scalar=bvg_t[:],
                        in1=xr_t[:, bass.ts(sg, NSG)],
                        op0=mybir.AluOpType.add, op1=mybir.AluOpType.add)
                    nc.sync.dma_start(out[:, bass.ts(sg, NSG)], o3[:])

    nc.compile()
    return nc


_NC_CACHE = None


def _get_nc():
    global _NC_CACHE
    if _NC_CACHE is None:
        _NC_CACHE = build_bass()
    return _NC_CACHE


def make_in_maps(x, Wq, bq, Wk, bk, Wv, bv, gamma):
    x = np.asarray(x, dtype=np.float32)
    Wq = np.asarray(Wq, dtype=np.float32)
    Wk = np.asarray(Wk, dtype=np.float32)
    Wv = np.asarray(Wv, dtype=np.float32)
    bq = np.asarray(bq, dtype=np.float32)
    bk = np.asarray(bk, dtype=np.float32)
    bv = np.asarray(bv, dtype=np.float32)
    gamma = np.asarray(gamma, dtype=np.float32)

    scale = np.float32(1.0 / np.sqrt(C))
    xf = x.reshape(B, C, N)
    wqt = np.ascontiguousarray((Wq * scale).T).astype(ml_dtypes.bfloat16)
    wkt = np.ascontiguousarray(Wk.T).astype(ml_dtypes.bfloat16)
    wvt = np.ascontiguousarray(Wv.T).astype(ml_dtypes.bfloat16)
    bq_s = (bq * scale).reshape(C, 1)
    bk_s = bk.reshape(C, 1).copy()
    g0 = np.float32(gamma.reshape(-1)[0])
    bvg_s = (bv.reshape(C, 1) * g0).astype(np.float32)
    gm_s = np.full((C, 1), g0, dtype=np.float32)

    in_maps = []
    for core in range(NCORES):
        b, h = core // 2, core % 2
        xslice = np.ascontiguousarray(xf[b][:, h * RQ:(h + 1) * RQ])
        in_maps.append({
            "xf": xf[b].astype(ml_dtypes.bfloat16),
            "xq": xslice.astype(ml_dtypes.bfloat16),
            "xr": xslice,
            "wqt": wqt, "wkt": wkt, "wvt": wvt,
            "bq": bq_s, "bk": bk_s, "bvg": bvg_s, "gm": gm_s,
        })
    return in_maps


def assemble(results):
    out = np.empty((B, C, N), dtype=np.float32)
    for core in range(NCORES):
        b, h = core // 2, core % 2
        out[b][:, h * RQ:(h + 1) * RQ] = results[core]["out"]
    return out.reshape(B, C, H, W)


def run(inputs: dict, trace: bool = False, tmpdir: str | None = None):
    nc = _get_nc()
    in_maps = make_in_maps(**inputs)
    res = run_bass_kernel_spmd(nc, in_maps, core_ids=list(range(NCORES)),
                               trace=trace, tmpdir=tmpdir)
    return assemble(res.results), res


def kernel(**inputs) -> np.ndarray:
    out, _ = run(inputs, trace=False)
    return out
